# revision 33
# baseline (speedup 1.0000x reference)
"""Trainium2 Bass kernel for nn_BaseGenerator (6-layer dense transformer).

Sharding: pure data-parallel over batch. Each of the 8 NeuronCores processes
8 of the 64 sequences end-to-end; no collectives. Host does embedding/bias
gathers (table lookups), layer-0 attention logits (f32; raw-embedding scores
reach +-2200 where bf16 would break softmax), the final edge scatter, and the
static -inf masks. The device runs the 6 transformer layers (bf16 matmuls,
f32 softmax/layernorm statistics), the final layernorm, the output heads and
the edge-logit einsum.

Self-contained: all shapes/layouts hardcoded.
"""
import numpy as np
import ml_dtypes

BF = ml_dtypes.bfloat16

# model dims
B, S = 64, 200
E, H, L, FF, HD = 512, 8, 6, 2048, 32
DH = E // H                  # 64
NCORES = 8
NB = B // NCORES             # 8 sequences per core
T = NB * S                   # 1600 tokens per core
TP = 1664                    # padded to 13*128
NTOK = TP // 128             # 13 token tiles
NKE = E // 128               # 4 k-tiles over E
NBLK = 4                     # n-blocks for feature-major matmuls
BLK = TP // NBLK             # 416
NFF = FF // 128              # 16
FFCH = [(0, 3), (3, 3), (6, 3), (9, 2), (11, 2)]  # FFN chunks (tok-tile start, count)
NEGV = np.float32(-1e30)
SCALE = np.float32(1.0 / np.sqrt(DH))
ESCALE = np.float32(HD ** -0.5)

_CACHE = {}


def _build_nc():
    import concourse.bass as bass
    import concourse.mybir as mybir
    import concourse.tile as tile
    from concourse.masks import make_identity

    f32 = mybir.dt.float32
    bf16 = mybir.dt.bfloat16
    AX = mybir.AxisListType.X
    AF = mybir.ActivationFunctionType
    OP = mybir.AluOpType

    nc = bass.Bass()

    # ---------------- DRAM parameters ----------------
    d_x0 = nc.declare_dram_parameter("x0", [128, NTOK, E], f32, isOutput=False)
    d_x0T = nc.declare_dram_parameter("x0T", [128, NKE, TP], bf16, isOutput=False)
    # layer-0 attention logits (host f32): [b][q%100, h, qt, k]
    d_t0 = nc.declare_dram_parameter("t0", [NB, 100, H, 2, S], f32, isOutput=False)
    # bias for layers 1..5 (bf16, pre-masked, same every layer): [b][q%100, h, qt, k]
    d_bias = nc.declare_dram_parameter("bias", [NB, 100, H, 2, S], bf16, isOutput=False)
    d_w = []
    for l in range(L):
        d_w.append(dict(
            wqk=nc.declare_dram_parameter(f"wqk{l}", [128, NKE, 2 * E], bf16, False),
            wv=nc.declare_dram_parameter(f"wv{l}", [128, NKE, E], bf16, False),
            wo=nc.declare_dram_parameter(f"wo{l}", [128, NKE, E], bf16, False),
            w1=nc.declare_dram_parameter(f"w1{l}", [128, NKE, FF], bf16, False),
            w2=nc.declare_dram_parameter(f"w2{l}", [128, NFF, E], bf16, False),
        ))
    d_whe = nc.declare_dram_parameter("whe", [128, NKE, 429], bf16, False)
    d_head = nc.declare_dram_parameter("head_out", [128, NTOK, 45], f32, isOutput=True)
    d_edge = nc.declare_dram_parameter("edge_out", [NB, 6, 100, 2, S], f32, isOutput=True)

    import bass_rust as _br

    def _split_excess_waits(ordered):
        """This toolchain's walrus accepts at most 1 sync wait per engine
        instruction; move extras onto preceding same-engine sequencer nops."""
        n = [0]
        for _, insts in ordered.items():
            out = []
            for inst in insts:
                try:
                    si = inst.sync_info
                except AttributeError:
                    si = None
                if si is not None and len(si.on_wait) > 1 and \
                        type(inst).__name__.startswith("Inst"):
                    waits = list(si.on_wait)
                    for w in waits[:-1]:
                        n[0] += 1
                        nop = _br.InstNoOp(name=f"WSPLIT-{n[0]}", engine=inst.engine)
                        nop.sync_info = mybir.SyncInfo(on_wait=[w], on_update=[])
                        out.append(nop)
                    inst.sync_info = mybir.SyncInfo(
                        on_wait=[waits[-1]], on_update=list(si.on_update))
                out.append(inst)
            insts[:] = out

    with tile.TileContext(nc) as tc:
        _orig_lower = tc._lower_ordered_insts

        def _patched_lower(ordered):
            _split_excess_waits(ordered)
            return _orig_lower(ordered)

        tc._lower_ordered_insts = _patched_lower

        from concourse.vector_clock import ScopedClock

        def _patched_drain_and_barrier(tick_clock, wait_clock):
            tmp = nc.sync.nop()
            wait_clock.add_sem_waits(
                tmp.ins, ScopedClock({None: tick_clock.global_clock}))
            si = tmp.ins.sync_info
            if si is not None and len(si.on_wait) > 1:
                waits = list(si.on_wait)
                tmp.ins.sync_info = mybir.SyncInfo(
                    on_wait=[waits[0]], on_update=list(si.on_update))
                for w in waits[1:]:
                    n2 = nc.sync.nop()
                    n2.ins.sync_info = mybir.SyncInfo(on_wait=[w], on_update=[])
            nc.sync.drain()
            nc.all_engine_barrier()
            popped = nc._tile_sem_poison_stack.pop()
            assert popped is tc._sem_poison
            nc.clear_and_free_semaphores(list(tc.sems.allocated().values()))
            nc.all_engine_barrier()

        tc._drain_and_barrier = _patched_drain_and_barrier
        import contextlib
        ctx = contextlib.ExitStack()
        with ctx:
            # ---------------- pools ----------------
            const_p = ctx.enter_context(tc.tile_pool(name="const", bufs=1))
            xp = ctx.enter_context(tc.tile_pool(name="xp", bufs=2))        # token-major x
            xTp = ctx.enter_context(tc.tile_pool(name="xTp", bufs=2))      # feature-major x
            qkTp = ctx.enter_context(tc.tile_pool(name="qkTp", bufs=1))
            vp = ctx.enter_context(tc.tile_pool(name="vp", bufs=1))
            oTp = ctx.enter_context(tc.tile_pool(name="oTp", bufs=1))
            hTp = ctx.enter_context(tc.tile_pool(name="hTp", bufs=1))      # FFN hidden chunk
            wqk_p = ctx.enter_context(tc.tile_pool(name="wqk", bufs=1))
            wv_p = ctx.enter_context(tc.tile_pool(name="wv", bufs=2))
            w1_p = ctx.enter_context(tc.tile_pool(name="w1", bufs=1))
            w2_p = ctx.enter_context(tc.tile_pool(name="w2", bufs=1))
            bt_p = ctx.enter_context(tc.tile_pool(name="bt", bufs=2))      # bias / t0 stream
            at_p = ctx.enter_context(tc.tile_pool(name="at", bufs=6))      # exp(t) tiles
            aT_p = ctx.enter_context(tc.tile_pool(name="aT", bufs=2))      # attn^T tiles
            sm_p = ctx.enter_context(tc.tile_pool(name="sm", bufs=3))      # small attn temps
            ln_p = ctx.enter_context(tc.tile_pool(name="ln", bufs=2))      # layernorm temps
            oute_p = ctx.enter_context(tc.tile_pool(name="oute", bufs=2))  # edge/head staging

            psMM = ctx.enter_context(tc.tile_pool(name="psMM", bufs=2, space="PSUM"))
            psA = ctx.enter_context(tc.tile_pool(name="psA", bufs=3, space="PSUM"))
            psAV = ctx.enter_context(tc.tile_pool(name="psAV", bufs=2, space="PSUM"))
            psT = ctx.enter_context(tc.tile_pool(name="psT", bufs=1, space="PSUM"))

            ident = const_p.tile([128, 128], bf16, tag="ident")
            make_identity(nc, ident)
            ident32 = const_p.tile([128, 128], f32, tag="ident32")
            make_identity(nc, ident32)
            eps_t = const_p.tile([128, 1], f32, tag="eps")
            nc.vector.memset(eps_t, 1e-5)

            # initial activations (x0T first: layer 0's matmuls need it;
            # x0 is only read at the out-projection residual)
            xT_cur = xTp.tile([128, NKE, TP], bf16, tag="xT")
            nc.sync.dma_start(out=xT_cur, in_=d_x0T[:])
            x_cur = xp.tile([128, NTOK, E], f32, tag="x")
            nc.sync.dma_start(out=x_cur, in_=d_x0[:])

            def ln_tile(psum_or_none, resid_slice, out_slice):
                """out = LN(psum + resid) (scale=1, bias=0). If psum is None, LN(resid)."""
                if psum_or_none is not None:
                    xr = psum_or_none
                    nc.vector.tensor_add(out=xr, in0=psum_or_none, in1=resid_slice)
                else:
                    xr = resid_slice
                st6 = ln_p.tile([128, 6], f32, tag="st6")
                nc.vector.bn_stats(out=st6, in_=xr)
                mv = ln_p.tile([128, 2], f32, tag="mv")
                nc.vector.bn_aggr(out=mv, in_=st6)
                rstd = ln_p.tile([128, 1], f32, tag="rstd")
                nc.scalar.activation(out=rstd, in_=mv[:, 1:2], func=AF.Sqrt, bias=eps_t)
                nc.vector.reciprocal(out=rstd, in_=rstd)
                nc.vector.tensor_scalar(
                    out=out_slice, in0=xr, scalar1=mv[:, 0:1], scalar2=rstd,
                    op0=OP.subtract, op1=OP.mult)

            def transpose_full(src_tok_major, dst_fmajor):
                # src is f32 token-major; dst is bf16 feature-major
                for mt in range(NTOK):
                    for kk in range(NKE):
                        ps = psT.tile([128, 128], f32, tag="ptr")
                        nc.tensor.transpose(ps, src_tok_major[:, mt, kk * 128:(kk + 1) * 128],
                                            ident32)
                        nc.any.tensor_copy(
                            out=dst_fmajor[:, kk, mt * 128:(mt + 1) * 128], in_=ps)

            for l in range(L):
                dw = d_w[l]
                # ---- load weights ----
                wv_sb = wv_p.tile([128, NKE, E], bf16, tag="wv")
                nc.sync.dma_start(out=wv_sb, in_=dw["wv"][:])
                wo_sb = wv_p.tile([128, NKE, E], bf16, tag="wv")
                nc.sync.dma_start(out=wo_sb, in_=dw["wo"][:])
                # ---- qkT (feature-major q,k) for layers >= 1 ----
                if l > 0:
                    wqk_sb = wqk_p.tile([128, NKE, 2 * E], bf16, tag="wqk")
                    nc.sync.dma_start(out=wqk_sb, in_=dw["wqk"][:])
                    qkT = qkTp.tile([128, 8, TP], bf16, tag="qkT")
                    for m in range(8):
                        for nb in range(NBLK):
                            ps = psMM.tile([128, 512], f32, tag="mm")
                            for kk in range(NKE):
                                nc.tensor.matmul(
                                    ps[:, 0:BLK],
                                    wqk_sb[:, kk, m * 128:(m + 1) * 128],
                                    xT_cur[:, kk, nb * BLK:(nb + 1) * BLK],
                                    start=(kk == 0), stop=(kk == NKE - 1))
                            nc.scalar.copy(
                                out=qkT[:, m, nb * BLK:(nb + 1) * BLK], in_=ps[:, 0:BLK])
                else:
                    qkT = None

                # ---- v (token-major, per-seq 100-tiles) ----
                v_sb = vp.tile([100, 2 * NB, E], bf16, tag="v")

                def v_tile(vt):
                    ps = psMM.tile([128, 512], f32, tag="mm", name=f"ps_v{vt}")
                    for kk in range(NKE):
                        nc.tensor.matmul(
                            ps[0:100, :],
                            xT_cur[:, kk, vt * 100:(vt + 1) * 100],
                            wv_sb[:, kk, :],
                            start=(kk == 0), stop=(kk == NKE - 1))
                    nc.scalar.copy(out=v_sb[:, vt, :], in_=ps[0:100, :])

                if l == 0:
                    # emit just the first sequence's v; the rest interleave
                    # with attention below to keep the PE warm in layer 0.
                    v_tile(0)
                    v_tile(1)
                else:
                    for vt in range(2 * NB):
                        v_tile(vt)

                # ---- attention (+ interleaved out-projection) ----
                oT = oTp.tile([128, NKE, TP], bf16, tag="oT")
                x_ln1 = xp.tile([128, NTOK, E], f32, tag="x")
                mv1 = ln_p.tile([128, NTOK, 2], f32, tag="mvall")

                def outproj_tile(mt):
                    ps = psMM.tile([128, 512], f32, tag="mm")
                    for kk in range(NKE):
                        nc.tensor.matmul(
                            ps, oT[:, kk, mt * 128:(mt + 1) * 128], wo_sb[:, kk, :],
                            start=(kk == 0), stop=(kk == NKE - 1))
                    # residual add straight into x_ln1 (unnormalized for now)
                    nc.vector.tensor_add(out=x_ln1[:, mt, :], in0=ps, in1=x_cur[:, mt, :])
                    st6 = ln_p.tile([128, 6], f32, tag="st6")
                    nc.vector.bn_stats(out=st6, in_=x_ln1[:, mt, :])
                    nc.vector.bn_aggr(out=mv1[:, mt, :], in_=st6)

                ready_mt = 0
                for b in range(NB):
                    if l == 0 and b < NB - 1:
                        v_tile(2 * b + 2)
                        v_tile(2 * b + 3)
                    for hh in range(4):     # group: heads 2*hh, 2*hh+1
                        if l == 0:
                            t0_b = bt_p.tile([100, 2, 2, S], f32, tag="bt")
                            nc.sync.dma_start(out=t0_b, in_=d_t0[b, :, 2 * hh:2 * hh + 2])
                            bias_b = None
                        else:
                            bias_b = bt_p.tile([100, 2, 2, S], bf16, tag="bt")
                            nc.sync.dma_start(out=bias_b, in_=d_bias[b, :, 2 * hh:2 * hh + 2])
                        den = sm_p.tile([100, 4], f32, tag="den")
                        rec = sm_p.tile([100, 4], f32, tag="rec")
                        ps_sc = {}
                        if l > 0:
                            for hi in range(2):
                                ps_sc[hi] = psA.tile([100, 2, S], f32, tag="sc",
                                                     name=f"ps_sc{hi}")
                            for qt in range(2):
                                for hi in range(2):   # alternate row groups for LDW overlap
                                    h = 2 * hh + hi
                                    po = hi * 64
                                    nc.tensor.matmul(
                                        ps_sc[hi][:, qt, :],
                                        qkT[po:po + 64, hh, b * S + qt * 100: b * S + qt * 100 + 100],
                                        qkT[po:po + 64, 4 + hh, b * S:(b + 1) * S],
                                        start=True, stop=True)
                            for hi in range(2):
                                nc.vector.scalar_tensor_tensor(
                                    out=ps_sc[hi], in0=ps_sc[hi], scalar=float(SCALE),
                                    in1=bias_b[:, hi], op0=OP.mult, op1=OP.add)
                        at_tiles = {}
                        for hi in range(2):
                            for qt in range(2):
                                src_t = t0_b[:, hi, qt, :] if l == 0 else ps_sc[hi][:, qt, :]
                                at = at_p.tile([100, S], bf16, tag="at")
                                nc.scalar.activation(
                                    out=at, in_=src_t, func=AF.Exp,
                                    accum_out=den[:, hi * 2 + qt: hi * 2 + qt + 1])
                                at_tiles[(hi, qt)] = at
                        nc.vector.reciprocal(out=rec, in_=den)
                        aT_tiles = {}
                        for hi in range(2):
                            aT = aT_p.tile([100, 2, S], bf16, tag="aT")
                            for qt in range(2):
                                atn = sm_p.tile([100, S], bf16, tag="atn")
                                nc.vector.tensor_scalar_mul(
                                    out=atn, in0=at_tiles[(hi, qt)],
                                    scalar1=rec[:, hi * 2 + qt: hi * 2 + qt + 1])
                                pst = psT.tile([100, 2, 100], bf16, tag="ptr")
                                nc.tensor.transpose(pst[:, 0, :], atn[:, 0:100], ident[0:100, 0:100])
                                nc.tensor.transpose(pst[:, 1, :], atn[:, 100:200], ident[0:100, 0:100])
                                nc.vector.tensor_copy(out=aT[:, :, qt * 100:(qt + 1) * 100], in_=pst)
                            aT_tiles[hi] = aT
                        ps_o = psAV.tile([128, S], f32, tag="av")
                        for sub in range(2):
                            h = 2 * hh + sub
                            for jt in range(2):
                                nc.tensor.matmul(
                                    ps_o[sub * 64:(sub + 1) * 64, :],
                                    v_sb[:, b * 2 + jt, h * 64:(h + 1) * 64],
                                    aT_tiles[sub][:, jt, :],
                                    start=(jt == 0), stop=(jt == 1),
                                    tile_position=(0, sub * 64))
                        nc.vector.tensor_copy(out=oT[:, hh, b * S:(b + 1) * S], in_=ps_o)
                    while ready_mt < NTOK and \
                            ((ready_mt + 1) * 128 <= (b + 1) * S or b == NB - 1):
                        outproj_tile(ready_mt)
                        ready_mt += 1

                w1_sb = w1_p.tile([128, NKE, FF], bf16, tag="w1")
                nc.sync.dma_start(out=w1_sb, in_=dw["w1"][:])
                w2_sb = w2_p.tile([128, NFF, E], bf16, tag="w2")
                nc.sync.dma_start(out=w2_sb, in_=dw["w2"][:])

                # ---- batched LN1 (two sqrt batches, then in-place normalize) ----
                rstd1 = ln_p.tile([128, NTOK], f32, tag="rstd")
                nc.scalar.activation(out=rstd1[:, 0:10], in_=mv1[:, 0:10, 1],
                                     func=AF.Sqrt, bias=eps_t)
                nc.scalar.activation(out=rstd1[:, 10:NTOK], in_=mv1[:, 10:NTOK, 1],
                                     func=AF.Sqrt, bias=eps_t)
                nc.vector.reciprocal(out=rstd1[:, 0:10], in_=rstd1[:, 0:10])
                nc.vector.reciprocal(out=rstd1[:, 10:NTOK], in_=rstd1[:, 10:NTOK])
                for mt in range(NTOK):
                    nc.vector.tensor_scalar(
                        out=x_ln1[:, mt, :], in0=x_ln1[:, mt, :],
                        scalar1=mv1[:, mt, 0:1], scalar2=rstd1[:, mt:mt + 1],
                        op0=OP.subtract, op1=OP.mult)

                # ---- transpose LN1 -> feature-major ----
                xT1 = xTp.tile([128, NKE, TP], bf16, tag="xT")
                transpose_full(x_ln1, xT1)

                # ---- FFN (chunked) + residual; LN2 batched at the end ----
                x_next = xp.tile([128, NTOK, E], f32, tag="x")
                mv2 = ln_p.tile([128, NTOK, 2], f32, tag="mvall")
                for (t0c, ntc) in FFCH:
                    cw = ntc * 128
                    hT = hTp.tile([128, NFF, 3 * 128], bf16, tag="hT")
                    for m in range(NFF):
                        ps = psMM.tile([128, 512], f32, tag="mm")
                        for kk in range(NKE):
                            nc.tensor.matmul(
                                ps[:, 0:cw],
                                w1_sb[:, kk, m * 128:(m + 1) * 128],
                                xT1[:, kk, t0c * 128: t0c * 128 + cw],
                                start=(kk == 0), stop=(kk == NKE - 1))
                        nc.scalar.activation(
                            out=hT[:, m, 0:cw], in_=ps[:, 0:cw], func=AF.Gelu)
                    for tt in range(ntc):
                        mt = t0c + tt
                        ps = psMM.tile([128, 512], f32, tag="mm")
                        for kk in range(NFF):
                            nc.tensor.matmul(
                                ps, hT[:, kk, tt * 128:(tt + 1) * 128], w2_sb[:, kk, :],
                                start=(kk == 0), stop=(kk == NFF - 1))
                        nc.vector.tensor_add(out=x_next[:, mt, :], in0=ps,
                                             in1=x_ln1[:, mt, :])
                        st6 = ln_p.tile([128, 6], f32, tag="st6")
                        nc.vector.bn_stats(out=st6, in_=x_next[:, mt, :])
                        nc.vector.bn_aggr(out=mv2[:, mt, :], in_=st6)
                rstd2 = ln_p.tile([128, NTOK], f32, tag="rstd")
                nc.scalar.activation(out=rstd2[:, 0:9], in_=mv2[:, 0:9, 1],
                                     func=AF.Sqrt, bias=eps_t)
                nc.scalar.activation(out=rstd2[:, 9:NTOK], in_=mv2[:, 9:NTOK, 1],
                                     func=AF.Sqrt, bias=eps_t)
                nc.vector.reciprocal(out=rstd2[:, 0:9], in_=rstd2[:, 0:9])
                nc.vector.reciprocal(out=rstd2[:, 9:NTOK], in_=rstd2[:, 9:NTOK])
                for mt in range(NTOK):
                    nc.vector.tensor_scalar(
                        out=x_next[:, mt, :], in0=x_next[:, mt, :],
                        scalar1=mv2[:, mt, 0:1], scalar2=rstd2[:, mt:mt + 1],
                        op0=OP.subtract, op1=OP.mult)

                # ---- transpose for next layer ----
                x_cur = x_next
                if l < L - 1:
                    xT_cur = xTp.tile([128, NKE, TP], bf16, tag="xT")
                    transpose_full(x_cur, xT_cur)

            # ---------------- final LN + heads ----------------
            x_fin = xp.tile([128, NTOK, E], f32, tag="x")
            mvf = ln_p.tile([128, NTOK, 2], f32, tag="mvall")
            for mt in range(NTOK):
                st6 = ln_p.tile([128, 6], f32, tag="st6")
                nc.vector.bn_stats(out=st6, in_=x_cur[:, mt, :])
                nc.vector.bn_aggr(out=mvf[:, mt, :], in_=st6)
            rstdf = ln_p.tile([128, NTOK], f32, tag="rstd")
            nc.scalar.activation(out=rstdf[:, 0:9], in_=mvf[:, 0:9, 1], func=AF.Sqrt,
                                 bias=eps_t)
            nc.scalar.activation(out=rstdf[:, 9:NTOK], in_=mvf[:, 9:NTOK, 1],
                                 func=AF.Sqrt, bias=eps_t)
            nc.vector.reciprocal(out=rstdf[:, 0:9], in_=rstdf[:, 0:9])
            nc.vector.reciprocal(out=rstdf[:, 9:NTOK], in_=rstdf[:, 9:NTOK])
            for mt in range(NTOK):
                nc.vector.tensor_scalar(
                    out=x_fin[:, mt, :], in0=x_cur[:, mt, :],
                    scalar1=mvf[:, mt, 0:1], scalar2=rstdf[:, mt:mt + 1],
                    op0=OP.subtract, op1=OP.mult)
            xTf = xTp.tile([128, NKE, TP], bf16, tag="xT")
            transpose_full(x_fin, xTf)

            # aob+atom logits: [tok, 45]
            whe_sb = wqk_p.tile([128, NKE, 429], bf16, tag="wqk")
            nc.sync.dma_start(out=whe_sb, in_=d_whe[:])
            for mt in range(NTOK):
                ps = psMM.tile([128, 512], f32, tag="mm")
                for kk in range(NKE):
                    nc.tensor.matmul(
                        ps[:, 0:45], xTf[:, kk, mt * 128:(mt + 1) * 128],
                        whe_sb[:, kk, 0:45],
                        start=(kk == 0), stop=(kk == NKE - 1))
                hd = oute_p.tile([128, 45], f32, tag="hd")
                nc.vector.tensor_copy(out=hd, in_=ps[:, 0:45])
                nc.sync.dma_start(out=d_head[:, mt, :], in_=hd)

            # edge projections out0/out1, feature-major [192(+pad), TP]
            e0T = qkTp.tile([128, 2, TP], bf16, tag="qkT")
            e1T = vp.tile([128, 2, TP], bf16, tag="v")

            def edge_einsum(b):
                for v in range(6):
                    vm, vo = (0, v * 32) if v < 4 else (1, (v - 4) * 32)
                    for st in range(2):
                        ed = oute_p.tile([100, S], f32, tag="edge", name=f"ed{b}_{v}_{st}")
                        ps_e = psA.tile([100, 2, S], f32, tag="sc", name=f"pse{b}_{v}_{st}")
                        nc.tensor.matmul(
                            ps_e[:, 0, :],
                            e0T[vo:vo + 32, vm, b * S + st * 100: b * S + st * 100 + 100],
                            e1T[vo:vo + 32, vm, b * S:(b + 1) * S],
                            start=True, stop=True,
                            tile_position=(vo, 0))
                        nc.vector.tensor_scalar_mul(out=ed, in0=ps_e[:, 0, :],
                                                    scalar1=float(ESCALE))
                        nc.sync.dma_start(out=d_edge[b, v, :, st, :], in_=ed)

            edge_ready = [2, 4, 6, 8]   # b's fully covered after nb blocks 0..3
            eb = 0
            for nb in range(NBLK):
                for which, eT in ((0, e0T), (1, e1T)):
                    base = 45 + which * 192
                    for m2, mw in ((0, 128), (1, 64)):
                        ps = psMM.tile([128, 512], f32, tag="mm",
                                       name=f"pse{nb}_{which}_{m2}")
                        for kk in range(NKE):
                            nc.tensor.matmul(
                                ps[0:mw, 0:BLK],
                                whe_sb[:, kk, base + m2 * 128: base + m2 * 128 + mw],
                                xTf[:, kk, nb * BLK:(nb + 1) * BLK],
                                start=(kk == 0), stop=(kk == NKE - 1))
                        nc.scalar.copy(
                            out=eT[0:mw, m2, nb * BLK:(nb + 1) * BLK], in_=ps[0:mw, 0:BLK])
                while eb < edge_ready[nb]:
                    edge_einsum(eb)
                    eb += 1

    return nc


def _host_pre(inputs):
    p = inputs["params"]

    def A(x):
        return np.asarray(x)

    aob = A(inputs["atom_or_bond_sequences"]).astype(np.int64)
    aid = A(inputs["atomid_sequences"]).astype(np.int64)
    bid = A(inputs["bondid_sequences"]).astype(np.int64)
    adj = A(inputs["adj_squares"]).astype(np.int64)
    aqs = A(inputs["atom_queue_id_squares"]).astype(np.int64)
    bqs = A(inputs["bond_queue_id_squares"]).astype(np.int64)

    sqrt_e = np.float32(np.sqrt(E))
    x0 = (A(p["tok_aob"])[aob] + A(p["tok_atom"])[aid] + A(p["tok_bond"])[bid]) * sqrt_e
    x0 = x0.astype(np.float32)  # (B,S,E)

    # bias (B,H,S,S) f32, masked
    g = A(p["adj_emb"])[adj] + A(p["atomq_emb"])[aqs] + A(p["bondq_emb"])[bqs]
    dist = np.abs(np.arange(S)[None, :] - np.arange(S)[:, None])
    g = g + A(p["dist_emb"])[dist][None]          # (B,S,S,H)
    bias = np.ascontiguousarray(g.transpose(0, 3, 1, 2)).astype(np.float32)
    # mask: (B,1,S,S) — broadcasts over H
    mask = (~np.tril(np.ones((S, S), bool)))[None, None] | (aob == 0)[:, None, None, :]
    bias = np.where(mask, NEGV, bias)

    # layer-0 attention logits on host (f32)
    w_in0 = A(p["layers"][0]["in_w"]).astype(np.float32)
    xf = x0.reshape(-1, E)
    q0 = (xf @ w_in0[:E].T).reshape(B, S, H, DH).transpose(0, 2, 1, 3)
    k0 = (xf @ w_in0[E:2 * E].T).reshape(B, S, H, DH).transpose(0, 2, 1, 3)
    t0 = np.matmul(q0, k0.transpose(0, 1, 3, 2)) * SCALE + bias   # (B,H,S,S)
    t0 = t0 - t0.max(axis=-1, keepdims=True)   # softmax max pre-subtracted on host

    def shuffle_bqhk(a, dt):
        # (B,H,S,S) -> per-core [NB,100,H,2,S]
        a = a.reshape(B, H, 2, 100, S).transpose(0, 3, 1, 2, 4)   # (B,100,H,2,S)
        a = np.ascontiguousarray(a).astype(dt)
        return a.reshape(NCORES, NB, 100, H, 2, S)

    t0_sh = shuffle_bqhk(t0, np.float32)
    bias_sh = shuffle_bqhk(bias, BF)

    # activations per core
    x0c = x0.reshape(NCORES, T, E)
    x0p = np.zeros((NCORES, TP, E), np.float32)
    x0p[:, :T] = x0c
    x0_tok = np.ascontiguousarray(
        x0p.reshape(NCORES, NTOK, 128, E).transpose(0, 2, 1, 3)).astype(np.float32)
    x0T = np.ascontiguousarray(
        x0p.transpose(0, 2, 1).reshape(NCORES, NKE, 128, TP).transpose(0, 2, 1, 3)).astype(BF)

    def wpack(w, nk):  # (out,in) f32 -> lhsT tiles [128, nk, out]
        wt = np.ascontiguousarray(w.T)             # (in, out)
        return np.ascontiguousarray(
            wt.reshape(nk, 128, w.shape[0]).transpose(1, 0, 2)).astype(BF)

    wmaps = {}
    for l in range(L):
        lp = p["layers"][l]
        in_w = A(lp["in_w"]).astype(np.float32)
        wmaps[f"wqk{l}"] = wpack(in_w[:2 * E], NKE)
        wmaps[f"wv{l}"] = wpack(in_w[2 * E:], NKE)
        wmaps[f"wo{l}"] = wpack(A(lp["out_w"]).astype(np.float32), NKE)
        wmaps[f"w1{l}"] = wpack(A(lp["w1"]).astype(np.float32), NKE)
        wmaps[f"w2{l}"] = wpack(A(lp["w2"]).astype(np.float32), NFF)
    whe = np.concatenate([A(p["gen_aob_w"]), A(p["gen_atom_w"]),
                          A(p["edge_w0"]), A(p["edge_w1"])], axis=0).astype(np.float32)
    wmaps["whe"] = wpack(whe, NKE)

    in_maps = []
    for c in range(NCORES):
        m = dict(wmaps)
        m["x0"] = x0_tok[c]
        m["x0T"] = x0T[c]
        m["t0"] = t0_sh[c]
        m["bias"] = bias_sh[c]
        in_maps.append(m)
    return in_maps, aqs


def _host_post(results, aqs):
    heads = []
    edges = []
    for c in range(NCORES):
        hd = results[c]["head_out"]                      # [128, NTOK, 45]
        hd = hd.transpose(1, 0, 2).reshape(TP, 45)[:T]   # (1600,45)
        heads.append(hd.reshape(NB, S, 45))
        ed = results[c]["edge_out"]                      # [NB,6,100,2,S]
        # logits_[b, s=st*100+p, t, v] = ed[b, v, p, st, t]
        edges.append(ed.transpose(0, 3, 2, 4, 1).reshape(NB, S, S, 6))
    head = np.concatenate(heads, 0)                      # (B,S,45)
    logits_ = np.concatenate(edges, 0).astype(np.float32)  # (B,S,S,6)

    aob_logits = np.ascontiguousarray(head[:, :, :5]).astype(np.float32)
    atom_logits = np.ascontiguousarray(head[:, :, 5:]).astype(np.float32)
    aob_logits[:, :, 0] = -np.inf
    aob_logits[:, :, 1] = -np.inf
    aob_logits[:, 0, 4] = -np.inf

    edge = np.full((B, S, 50, 6), -np.inf, np.float32)
    bi = np.arange(B)[:, None, None]
    si = np.arange(S)[None, :, None]
    edge[bi, si, aqs] = logits_
    edge[:, :, 0, :] = -np.inf
    edge = edge.reshape(B, S, 300)
    return aob_logits, atom_logits, edge


TRACE = False       # set True (e.g. from test.py) to capture a neuron profile


def kernel(**inputs):
    from concourse.bass_utils import run_bass_kernel_spmd
    if "nc" not in _CACHE:
        _CACHE["nc"] = _build_nc()
    nc = _CACHE["nc"]
    in_maps, aqs = _host_pre(inputs)
    res = run_bass_kernel_spmd(nc, in_maps, list(range(NCORES)), trace=TRACE)
    _CACHE["last"] = res
    return _host_post(res.results, aqs)


# revision 34
# speedup vs baseline: 1.0872x; 1.0872x over previous
"""Trainium2 Bass kernel for nn_BaseGenerator (6-layer dense transformer).

Sharding: pure data-parallel over batch. Each of the 8 NeuronCores processes
8 of the 64 sequences end-to-end; no collectives. Host does embedding/bias
gathers (table lookups), layer-0 attention logits (f32; raw-embedding scores
reach +-2200 where bf16 would break softmax), the final edge scatter, and the
static -inf masks. The device runs the 6 transformer layers (bf16 matmuls,
f32 softmax/layernorm statistics), the final layernorm, the output heads and
the edge-logit einsum.

Self-contained: all shapes/layouts hardcoded.
"""
import numpy as np
import ml_dtypes

BF = ml_dtypes.bfloat16

# model dims
B, S = 64, 200
E, H, L, FF, HD = 512, 8, 6, 2048, 32
DH = E // H                  # 64
NCORES = 8
NB = B // NCORES             # 8 sequences per core
T = NB * S                   # 1600 tokens per core
TP = 1664                    # padded to 13*128
NTOK = TP // 128             # 13 token tiles
NKE = E // 128               # 4 k-tiles over E
NBLK = 4                     # n-blocks for feature-major matmuls
BLK = TP // NBLK             # 416
NFF = FF // 128              # 16
FFCH = [(0, 3), (3, 3), (6, 3), (9, 2), (11, 2)]  # FFN chunks (tok-tile start, count)
NEGV = np.float32(-1e30)
SCALE = np.float32(1.0 / np.sqrt(DH))
ESCALE = np.float32(HD ** -0.5)

_CACHE = {}


def _build_nc():
    import concourse.bass as bass
    import concourse.mybir as mybir
    import concourse.tile as tile
    from concourse.masks import make_identity

    f32 = mybir.dt.float32
    bf16 = mybir.dt.bfloat16
    AX = mybir.AxisListType.X
    AF = mybir.ActivationFunctionType
    OP = mybir.AluOpType

    nc = bass.Bass()

    # ---------------- DRAM parameters ----------------
    d_x0 = nc.declare_dram_parameter("x0", [128, NTOK, E], f32, isOutput=False)
    d_x0T = nc.declare_dram_parameter("x0T", [128, NKE, TP], bf16, isOutput=False)
    # layer-0 attention logits (host f32): [b][q%100, h, qt, k]
    d_t0 = nc.declare_dram_parameter("t0", [NB, 100, H, 2, S], f32, isOutput=False)
    # bias for layers 1..5 (bf16, pre-masked, same every layer): [b][q%100, h, qt, k]
    d_bias = nc.declare_dram_parameter("bias", [NB, 100, H, 2, S], bf16, isOutput=False)
    d_w = []
    for l in range(L):
        d_w.append(dict(
            wqk=nc.declare_dram_parameter(f"wqk{l}", [128, NKE, 2 * E], bf16, False),
            wv=nc.declare_dram_parameter(f"wv{l}", [128, NKE, E], bf16, False),
            wo=nc.declare_dram_parameter(f"wo{l}", [128, NKE, E], bf16, False),
            w1=nc.declare_dram_parameter(f"w1{l}", [128, NKE, FF], bf16, False),
            w2=nc.declare_dram_parameter(f"w2{l}", [128, NFF, E], bf16, False),
        ))
    d_whe = nc.declare_dram_parameter("whe", [128, NKE, 429], bf16, False)
    d_head = nc.declare_dram_parameter("head_out", [128, NTOK, 45], f32, isOutput=True)
    d_edge = nc.declare_dram_parameter("edge_out", [NB, 6, 100, 2, S], f32, isOutput=True)

    import bass_rust as _br

    def _split_excess_waits(ordered):
        """This toolchain's walrus accepts at most 1 sync wait per engine
        instruction; move extras onto preceding same-engine sequencer nops."""
        n = [0]
        for _, insts in ordered.items():
            out = []
            for inst in insts:
                try:
                    si = inst.sync_info
                except AttributeError:
                    si = None
                if si is not None and len(si.on_wait) > 1 and \
                        type(inst).__name__.startswith("Inst"):
                    waits = list(si.on_wait)
                    for w in waits[:-1]:
                        n[0] += 1
                        nop = _br.InstNoOp(name=f"WSPLIT-{n[0]}", engine=inst.engine)
                        nop.sync_info = mybir.SyncInfo(on_wait=[w], on_update=[])
                        out.append(nop)
                    inst.sync_info = mybir.SyncInfo(
                        on_wait=[waits[-1]], on_update=list(si.on_update))
                out.append(inst)
            insts[:] = out

    with tile.TileContext(nc) as tc:
        _orig_lower = tc._lower_ordered_insts

        def _patched_lower(ordered):
            _split_excess_waits(ordered)
            return _orig_lower(ordered)

        tc._lower_ordered_insts = _patched_lower

        from concourse.vector_clock import ScopedClock

        def _patched_drain_and_barrier(tick_clock, wait_clock):
            tmp = nc.sync.nop()
            wait_clock.add_sem_waits(
                tmp.ins, ScopedClock({None: tick_clock.global_clock}))
            si = tmp.ins.sync_info
            if si is not None and len(si.on_wait) > 1:
                waits = list(si.on_wait)
                tmp.ins.sync_info = mybir.SyncInfo(
                    on_wait=[waits[0]], on_update=list(si.on_update))
                for w in waits[1:]:
                    n2 = nc.sync.nop()
                    n2.ins.sync_info = mybir.SyncInfo(on_wait=[w], on_update=[])
            nc.sync.drain()
            nc.all_engine_barrier()
            popped = nc._tile_sem_poison_stack.pop()
            assert popped is tc._sem_poison
            nc.clear_and_free_semaphores(list(tc.sems.allocated().values()))
            nc.all_engine_barrier()

        tc._drain_and_barrier = _patched_drain_and_barrier
        import contextlib
        ctx = contextlib.ExitStack()
        with ctx:
            # ---------------- pools ----------------
            const_p = ctx.enter_context(tc.tile_pool(name="const", bufs=1))
            xp = ctx.enter_context(tc.tile_pool(name="xp", bufs=2))        # token-major x
            xTp = ctx.enter_context(tc.tile_pool(name="xTp", bufs=2))      # feature-major x
            qkTp = ctx.enter_context(tc.tile_pool(name="qkTp", bufs=1))
            vp = ctx.enter_context(tc.tile_pool(name="vp", bufs=1))
            oTp = ctx.enter_context(tc.tile_pool(name="oTp", bufs=1))
            hTp = ctx.enter_context(tc.tile_pool(name="hTp", bufs=1))      # FFN hidden chunk
            wqk_p = ctx.enter_context(tc.tile_pool(name="wqk", bufs=1))
            wv_p = ctx.enter_context(tc.tile_pool(name="wv", bufs=2))
            w1_p = ctx.enter_context(tc.tile_pool(name="w1", bufs=1))
            w2_p = ctx.enter_context(tc.tile_pool(name="w2", bufs=1))
            bt_p = ctx.enter_context(tc.tile_pool(name="bt", bufs=2))      # bias / t0 stream
            at_p = ctx.enter_context(tc.tile_pool(name="at", bufs=6))      # exp(t) tiles
            aT_p = ctx.enter_context(tc.tile_pool(name="aT", bufs=2))      # attn^T tiles
            sm_p = ctx.enter_context(tc.tile_pool(name="sm", bufs=3))      # small attn temps
            ln_p = ctx.enter_context(tc.tile_pool(name="ln", bufs=2))      # layernorm temps
            oute_p = ctx.enter_context(tc.tile_pool(name="oute", bufs=2))  # edge/head staging

            psMM = ctx.enter_context(tc.tile_pool(name="psMM", bufs=2, space="PSUM"))
            psA = ctx.enter_context(tc.tile_pool(name="psA", bufs=2, space="PSUM"))
            psAV = ctx.enter_context(tc.tile_pool(name="psAV", bufs=2, space="PSUM"))
            psT = ctx.enter_context(tc.tile_pool(name="psT", bufs=2, space="PSUM"))

            ident = const_p.tile([128, 128], bf16, tag="ident")
            make_identity(nc, ident)
            ident32 = const_p.tile([128, 128], f32, tag="ident32")
            make_identity(nc, ident32)
            eps_t = const_p.tile([128, 1], f32, tag="eps")
            nc.vector.memset(eps_t, 1e-5)

            # initial activations (x0T first: layer 0's matmuls need it;
            # x0 is only read at the out-projection residual)
            xT_cur = xTp.tile([128, NKE, TP], bf16, tag="xT")
            nc.sync.dma_start(out=xT_cur, in_=d_x0T[:])
            x_cur = xp.tile([128, NTOK, E], f32, tag="x")
            nc.sync.dma_start(out=x_cur, in_=d_x0[:])

            def ln_tile(psum_or_none, resid_slice, out_slice):
                """out = LN(psum + resid) (scale=1, bias=0). If psum is None, LN(resid)."""
                if psum_or_none is not None:
                    xr = psum_or_none
                    nc.vector.tensor_add(out=xr, in0=psum_or_none, in1=resid_slice)
                else:
                    xr = resid_slice
                st6 = ln_p.tile([128, 6], f32, tag="st6")
                nc.vector.bn_stats(out=st6, in_=xr)
                mv = ln_p.tile([128, 2], f32, tag="mv")
                nc.vector.bn_aggr(out=mv, in_=st6)
                rstd = ln_p.tile([128, 1], f32, tag="rstd")
                nc.scalar.activation(out=rstd, in_=mv[:, 1:2], func=AF.Sqrt, bias=eps_t)
                nc.vector.reciprocal(out=rstd, in_=rstd)
                nc.vector.tensor_scalar(
                    out=out_slice, in0=xr, scalar1=mv[:, 0:1], scalar2=rstd,
                    op0=OP.subtract, op1=OP.mult)

            def transpose_full(src_tok_major, dst_fmajor):
                # src is f32 token-major; dst is bf16 feature-major
                for mt in range(NTOK):
                    for kk in range(NKE):
                        ps = psT.tile([128, 128], f32, tag="ptr")
                        nc.tensor.transpose(ps, src_tok_major[:, mt, kk * 128:(kk + 1) * 128],
                                            ident32)
                        nc.any.tensor_copy(
                            out=dst_fmajor[:, kk, mt * 128:(mt + 1) * 128], in_=ps)

            for l in range(L):
                dw = d_w[l]
                # ---- load weights ----
                wv_sb = wv_p.tile([128, NKE, E], bf16, tag="wv")
                nc.sync.dma_start(out=wv_sb, in_=dw["wv"][:])
                wo_sb = wv_p.tile([128, NKE, E], bf16, tag="wv")
                nc.sync.dma_start(out=wo_sb, in_=dw["wo"][:])
                # ---- qkT (feature-major q,k) for layers >= 1 ----
                if l > 0:
                    wqk_sb = wqk_p.tile([128, NKE, 2 * E], bf16, tag="wqk")
                    nc.sync.dma_start(out=wqk_sb, in_=dw["wqk"][:])
                    qkT = qkTp.tile([128, 8, TP], bf16, tag="qkT")
                    for m in range(8):
                        for nb in range(NBLK):
                            ps = psMM.tile([128, 512], f32, tag="mm")
                            for kk in range(NKE):
                                nc.tensor.matmul(
                                    ps[:, 0:BLK],
                                    wqk_sb[:, kk, m * 128:(m + 1) * 128],
                                    xT_cur[:, kk, nb * BLK:(nb + 1) * BLK],
                                    start=(kk == 0), stop=(kk == NKE - 1))
                            nc.scalar.copy(
                                out=qkT[:, m, nb * BLK:(nb + 1) * BLK], in_=ps[:, 0:BLK])
                else:
                    qkT = None

                # ---- v (token-major, per-seq 100-tiles) ----
                v_sb = vp.tile([100, 2 * NB, E], bf16, tag="v")

                def v_tile(vt):
                    ps = psMM.tile([128, 512], f32, tag="mm", name=f"ps_v{vt}")
                    for kk in range(NKE):
                        nc.tensor.matmul(
                            ps[0:100, :],
                            xT_cur[:, kk, vt * 100:(vt + 1) * 100],
                            wv_sb[:, kk, :],
                            start=(kk == 0), stop=(kk == NKE - 1))
                    nc.scalar.copy(out=v_sb[:, vt, :], in_=ps[0:100, :])

                if l == 0:
                    # emit just the first sequence's v; the rest interleave
                    # with attention below to keep the PE warm in layer 0.
                    v_tile(0)
                    v_tile(1)
                else:
                    for vt in range(2 * NB):
                        v_tile(vt)

                # ---- attention (+ interleaved out-projection) ----
                oT = oTp.tile([128, NKE, TP], bf16, tag="oT")
                x_ln1 = xp.tile([128, NTOK, E], f32, tag="x")
                mv1 = ln_p.tile([128, NTOK, 2], f32, tag="mvall")

                def outproj_tile(mt):
                    ps = psMM.tile([128, 512], f32, tag="mm")
                    for kk in range(NKE):
                        nc.tensor.matmul(
                            ps, oT[:, kk, mt * 128:(mt + 1) * 128], wo_sb[:, kk, :],
                            start=(kk == 0), stop=(kk == NKE - 1))
                    # residual add straight into x_ln1 (unnormalized for now)
                    nc.vector.tensor_add(out=x_ln1[:, mt, :], in0=ps, in1=x_cur[:, mt, :])
                    st6 = ln_p.tile([128, 6], f32, tag="st6")
                    nc.vector.bn_stats(out=st6, in_=x_ln1[:, mt, :])
                    nc.vector.bn_aggr(out=mv1[:, mt, :], in_=st6)

                ready_mt = 0
                for b in range(NB):
                    if l == 0 and b < NB - 1:
                        v_tile(2 * b + 2)
                        v_tile(2 * b + 3)
                    for hh in range(4):     # group: heads 2*hh, 2*hh+1
                        if l == 0:
                            t0_b = bt_p.tile([100, 2, 2, S], f32, tag="bt")
                            nc.sync.dma_start(out=t0_b, in_=d_t0[b, :, 2 * hh:2 * hh + 2])
                            bias_b = None
                        else:
                            bias_b = bt_p.tile([100, 2, 2, S], bf16, tag="bt")
                            nc.sync.dma_start(out=bias_b, in_=d_bias[b, :, 2 * hh:2 * hh + 2])
                        den = sm_p.tile([100, 4], f32, tag="den")
                        rec = sm_p.tile([100, 4], f32, tag="rec")
                        ps_sc = {}
                        if l > 0:
                            for hi in range(2):
                                ps_sc[hi] = psA.tile([100, 2, S], f32, tag="sc",
                                                     name=f"ps_sc{hi}")
                            for qt in range(2):
                                for hi in range(2):   # alternate row groups for LDW overlap
                                    h = 2 * hh + hi
                                    po = hi * 64
                                    nc.tensor.matmul(
                                        ps_sc[hi][:, qt, :],
                                        qkT[po:po + 64, hh, b * S + qt * 100: b * S + qt * 100 + 100],
                                        qkT[po:po + 64, 4 + hh, b * S:(b + 1) * S],
                                        start=True, stop=True)
                            for hi in range(2):
                                nc.vector.scalar_tensor_tensor(
                                    out=ps_sc[hi], in0=ps_sc[hi], scalar=float(SCALE),
                                    in1=bias_b[:, hi], op0=OP.mult, op1=OP.add)
                        at_tiles = {}
                        for hi in range(2):
                            for qt in range(2):
                                src_t = t0_b[:, hi, qt, :] if l == 0 else ps_sc[hi][:, qt, :]
                                at = at_p.tile([100, S], bf16, tag="at")
                                nc.scalar.activation(
                                    out=at, in_=src_t, func=AF.Exp,
                                    accum_out=den[:, hi * 2 + qt: hi * 2 + qt + 1])
                                at_tiles[(hi, qt)] = at
                        nc.vector.reciprocal(out=rec, in_=den)
                        aT_tiles = {}
                        for hi in range(2):
                            aT = aT_p.tile([100, 2, S], bf16, tag="aT")
                            for qt in range(2):
                                atn = sm_p.tile([100, S], bf16, tag="atn")
                                nc.vector.tensor_scalar_mul(
                                    out=atn, in0=at_tiles[(hi, qt)],
                                    scalar1=rec[:, hi * 2 + qt: hi * 2 + qt + 1])
                                pst = psT.tile([100, 2, 100], bf16, tag="ptr")
                                nc.tensor.transpose(pst[:, 0, :], atn[:, 0:100], ident[0:100, 0:100])
                                nc.tensor.transpose(pst[:, 1, :], atn[:, 100:200], ident[0:100, 0:100])
                                nc.vector.tensor_copy(out=aT[:, :, qt * 100:(qt + 1) * 100], in_=pst)
                            aT_tiles[hi] = aT
                        ps_o = psAV.tile([128, S], f32, tag="av")
                        for sub in range(2):
                            h = 2 * hh + sub
                            for jt in range(2):
                                nc.tensor.matmul(
                                    ps_o[sub * 64:(sub + 1) * 64, :],
                                    v_sb[:, b * 2 + jt, h * 64:(h + 1) * 64],
                                    aT_tiles[sub][:, jt, :],
                                    start=(jt == 0), stop=(jt == 1),
                                    tile_position=(0, sub * 64))
                        nc.vector.tensor_copy(out=oT[:, hh, b * S:(b + 1) * S], in_=ps_o)
                    while ready_mt < NTOK and \
                            ((ready_mt + 1) * 128 <= (b + 1) * S or b == NB - 1):
                        outproj_tile(ready_mt)
                        ready_mt += 1

                w1_sb = w1_p.tile([128, NKE, FF], bf16, tag="w1")
                nc.sync.dma_start(out=w1_sb, in_=dw["w1"][:])
                w2_sb = w2_p.tile([128, NFF, E], bf16, tag="w2")
                nc.sync.dma_start(out=w2_sb, in_=dw["w2"][:])

                # ---- batched LN1 (two sqrt batches, then in-place normalize) ----
                rstd1 = ln_p.tile([128, NTOK], f32, tag="rstd")
                nc.scalar.activation(out=rstd1[:, 0:10], in_=mv1[:, 0:10, 1],
                                     func=AF.Sqrt, bias=eps_t)
                nc.scalar.activation(out=rstd1[:, 10:NTOK], in_=mv1[:, 10:NTOK, 1],
                                     func=AF.Sqrt, bias=eps_t)
                nc.vector.reciprocal(out=rstd1[:, 0:10], in_=rstd1[:, 0:10])
                nc.vector.reciprocal(out=rstd1[:, 10:NTOK], in_=rstd1[:, 10:NTOK])
                for mt in range(NTOK):
                    nc.vector.tensor_scalar(
                        out=x_ln1[:, mt, :], in0=x_ln1[:, mt, :],
                        scalar1=mv1[:, mt, 0:1], scalar2=rstd1[:, mt:mt + 1],
                        op0=OP.subtract, op1=OP.mult)

                # ---- transpose LN1 -> feature-major ----
                xT1 = xTp.tile([128, NKE, TP], bf16, tag="xT")
                transpose_full(x_ln1, xT1)

                # ---- FFN (chunked) + residual; LN2 batched at the end ----
                x_next = xp.tile([128, NTOK, E], f32, tag="x")
                mv2 = ln_p.tile([128, NTOK, 2], f32, tag="mvall")
                for (t0c, ntc) in FFCH:
                    cw = ntc * 128
                    hT = hTp.tile([128, NFF, 3 * 128], bf16, tag="hT")
                    for m in range(NFF):
                        ps = psMM.tile([128, 512], f32, tag="mm")
                        for kk in range(NKE):
                            nc.tensor.matmul(
                                ps[:, 0:cw],
                                w1_sb[:, kk, m * 128:(m + 1) * 128],
                                xT1[:, kk, t0c * 128: t0c * 128 + cw],
                                start=(kk == 0), stop=(kk == NKE - 1))
                        nc.scalar.activation(
                            out=hT[:, m, 0:cw], in_=ps[:, 0:cw], func=AF.Gelu)
                    for tt in range(ntc):
                        mt = t0c + tt
                        ps = psMM.tile([128, 512], f32, tag="mm")
                        for kk in range(NFF):
                            nc.tensor.matmul(
                                ps, hT[:, kk, tt * 128:(tt + 1) * 128], w2_sb[:, kk, :],
                                start=(kk == 0), stop=(kk == NFF - 1))
                        nc.vector.tensor_add(out=x_next[:, mt, :], in0=ps,
                                             in1=x_ln1[:, mt, :])
                        st6 = ln_p.tile([128, 6], f32, tag="st6")
                        nc.vector.bn_stats(out=st6, in_=x_next[:, mt, :])
                        nc.vector.bn_aggr(out=mv2[:, mt, :], in_=st6)
                rstd2 = ln_p.tile([128, NTOK], f32, tag="rstd")
                nc.scalar.activation(out=rstd2[:, 0:9], in_=mv2[:, 0:9, 1],
                                     func=AF.Sqrt, bias=eps_t)
                nc.scalar.activation(out=rstd2[:, 9:NTOK], in_=mv2[:, 9:NTOK, 1],
                                     func=AF.Sqrt, bias=eps_t)
                nc.vector.reciprocal(out=rstd2[:, 0:9], in_=rstd2[:, 0:9])
                nc.vector.reciprocal(out=rstd2[:, 9:NTOK], in_=rstd2[:, 9:NTOK])
                for mt in range(NTOK):
                    nc.vector.tensor_scalar(
                        out=x_next[:, mt, :], in0=x_next[:, mt, :],
                        scalar1=mv2[:, mt, 0:1], scalar2=rstd2[:, mt:mt + 1],
                        op0=OP.subtract, op1=OP.mult)

                # ---- transpose for next layer ----
                x_cur = x_next
                if l < L - 1:
                    xT_cur = xTp.tile([128, NKE, TP], bf16, tag="xT")
                    transpose_full(x_cur, xT_cur)

            # ---------------- final LN + heads ----------------
            x_fin = xp.tile([128, NTOK, E], f32, tag="x")
            mvf = ln_p.tile([128, NTOK, 2], f32, tag="mvall")
            for mt in range(NTOK):
                st6 = ln_p.tile([128, 6], f32, tag="st6")
                nc.vector.bn_stats(out=st6, in_=x_cur[:, mt, :])
                nc.vector.bn_aggr(out=mvf[:, mt, :], in_=st6)
            rstdf = ln_p.tile([128, NTOK], f32, tag="rstd")
            nc.scalar.activation(out=rstdf[:, 0:9], in_=mvf[:, 0:9, 1], func=AF.Sqrt,
                                 bias=eps_t)
            nc.scalar.activation(out=rstdf[:, 9:NTOK], in_=mvf[:, 9:NTOK, 1],
                                 func=AF.Sqrt, bias=eps_t)
            nc.vector.reciprocal(out=rstdf[:, 0:9], in_=rstdf[:, 0:9])
            nc.vector.reciprocal(out=rstdf[:, 9:NTOK], in_=rstdf[:, 9:NTOK])
            for mt in range(NTOK):
                nc.vector.tensor_scalar(
                    out=x_fin[:, mt, :], in0=x_cur[:, mt, :],
                    scalar1=mvf[:, mt, 0:1], scalar2=rstdf[:, mt:mt + 1],
                    op0=OP.subtract, op1=OP.mult)
            xTf = xTp.tile([128, NKE, TP], bf16, tag="xT")
            transpose_full(x_fin, xTf)

            # aob+atom logits: [tok, 45]
            whe_sb = wqk_p.tile([128, NKE, 429], bf16, tag="wqk")
            nc.sync.dma_start(out=whe_sb, in_=d_whe[:])
            for mt in range(NTOK):
                ps = psMM.tile([128, 512], f32, tag="mm")
                for kk in range(NKE):
                    nc.tensor.matmul(
                        ps[:, 0:45], xTf[:, kk, mt * 128:(mt + 1) * 128],
                        whe_sb[:, kk, 0:45],
                        start=(kk == 0), stop=(kk == NKE - 1))
                hd = oute_p.tile([128, 45], f32, tag="hd")
                nc.vector.tensor_copy(out=hd, in_=ps[:, 0:45])
                nc.sync.dma_start(out=d_head[:, mt, :], in_=hd)

            # edge projections out0/out1, feature-major [192(+pad), TP]
            e0T = qkTp.tile([128, 2, TP], bf16, tag="qkT")
            e1T = vp.tile([128, 2, TP], bf16, tag="v")

            def edge_einsum(b):
                for v in range(6):
                    vm, vo = (0, v * 32) if v < 4 else (1, (v - 4) * 32)
                    for st in range(2):
                        ed = oute_p.tile([100, S], f32, tag="edge", name=f"ed{b}_{v}_{st}")
                        ps_e = psA.tile([100, 2, S], f32, tag="sc", name=f"pse{b}_{v}_{st}")
                        nc.tensor.matmul(
                            ps_e[:, 0, :],
                            e0T[vo:vo + 32, vm, b * S + st * 100: b * S + st * 100 + 100],
                            e1T[vo:vo + 32, vm, b * S:(b + 1) * S],
                            start=True, stop=True,
                            tile_position=(vo, 0))
                        nc.vector.tensor_scalar_mul(out=ed, in0=ps_e[:, 0, :],
                                                    scalar1=float(ESCALE))
                        nc.sync.dma_start(out=d_edge[b, v, :, st, :], in_=ed)

            edge_ready = [2, 4, 6, 8]   # b's fully covered after nb blocks 0..3
            eb = 0
            for nb in range(NBLK):
                for which, eT in ((0, e0T), (1, e1T)):
                    base = 45 + which * 192
                    for m2, mw in ((0, 128), (1, 64)):
                        ps = psMM.tile([128, 512], f32, tag="mm",
                                       name=f"pse{nb}_{which}_{m2}")
                        for kk in range(NKE):
                            nc.tensor.matmul(
                                ps[0:mw, 0:BLK],
                                whe_sb[:, kk, base + m2 * 128: base + m2 * 128 + mw],
                                xTf[:, kk, nb * BLK:(nb + 1) * BLK],
                                start=(kk == 0), stop=(kk == NKE - 1))
                        nc.scalar.copy(
                            out=eT[0:mw, m2, nb * BLK:(nb + 1) * BLK], in_=ps[0:mw, 0:BLK])
                while eb < edge_ready[nb]:
                    edge_einsum(eb)
                    eb += 1

    return nc


def _host_pre(inputs):
    p = inputs["params"]

    def A(x):
        return np.asarray(x)

    aob = A(inputs["atom_or_bond_sequences"]).astype(np.int64)
    aid = A(inputs["atomid_sequences"]).astype(np.int64)
    bid = A(inputs["bondid_sequences"]).astype(np.int64)
    adj = A(inputs["adj_squares"]).astype(np.int64)
    aqs = A(inputs["atom_queue_id_squares"]).astype(np.int64)
    bqs = A(inputs["bond_queue_id_squares"]).astype(np.int64)

    sqrt_e = np.float32(np.sqrt(E))
    x0 = (A(p["tok_aob"])[aob] + A(p["tok_atom"])[aid] + A(p["tok_bond"])[bid]) * sqrt_e
    x0 = x0.astype(np.float32)  # (B,S,E)

    # bias (B,H,S,S) f32, masked
    g = A(p["adj_emb"])[adj] + A(p["atomq_emb"])[aqs] + A(p["bondq_emb"])[bqs]
    dist = np.abs(np.arange(S)[None, :] - np.arange(S)[:, None])
    g = g + A(p["dist_emb"])[dist][None]          # (B,S,S,H)
    bias = np.ascontiguousarray(g.transpose(0, 3, 1, 2)).astype(np.float32)
    # mask: (B,1,S,S) — broadcasts over H
    mask = (~np.tril(np.ones((S, S), bool)))[None, None] | (aob == 0)[:, None, None, :]
    bias = np.where(mask, NEGV, bias)

    # layer-0 attention logits on host (f32)
    w_in0 = A(p["layers"][0]["in_w"]).astype(np.float32)
    xf = x0.reshape(-1, E)
    q0 = (xf @ w_in0[:E].T).reshape(B, S, H, DH).transpose(0, 2, 1, 3)
    k0 = (xf @ w_in0[E:2 * E].T).reshape(B, S, H, DH).transpose(0, 2, 1, 3)
    t0 = np.matmul(q0, k0.transpose(0, 1, 3, 2)) * SCALE + bias   # (B,H,S,S)
    t0 = t0 - t0.max(axis=-1, keepdims=True)   # softmax max pre-subtracted on host

    def shuffle_bqhk(a, dt):
        # (B,H,S,S) -> per-core [NB,100,H,2,S]
        a = a.reshape(B, H, 2, 100, S).transpose(0, 3, 1, 2, 4)   # (B,100,H,2,S)
        a = np.ascontiguousarray(a).astype(dt)
        return a.reshape(NCORES, NB, 100, H, 2, S)

    t0_sh = shuffle_bqhk(t0, np.float32)
    bias_sh = shuffle_bqhk(bias, BF)

    # activations per core
    x0c = x0.reshape(NCORES, T, E)
    x0p = np.zeros((NCORES, TP, E), np.float32)
    x0p[:, :T] = x0c
    x0_tok = np.ascontiguousarray(
        x0p.reshape(NCORES, NTOK, 128, E).transpose(0, 2, 1, 3)).astype(np.float32)
    x0T = np.ascontiguousarray(
        x0p.transpose(0, 2, 1).reshape(NCORES, NKE, 128, TP).transpose(0, 2, 1, 3)).astype(BF)

    def wpack(w, nk):  # (out,in) f32 -> lhsT tiles [128, nk, out]
        wt = np.ascontiguousarray(w.T)             # (in, out)
        return np.ascontiguousarray(
            wt.reshape(nk, 128, w.shape[0]).transpose(1, 0, 2)).astype(BF)

    wmaps = {}
    for l in range(L):
        lp = p["layers"][l]
        in_w = A(lp["in_w"]).astype(np.float32)
        wmaps[f"wqk{l}"] = wpack(in_w[:2 * E], NKE)
        wmaps[f"wv{l}"] = wpack(in_w[2 * E:], NKE)
        wmaps[f"wo{l}"] = wpack(A(lp["out_w"]).astype(np.float32), NKE)
        wmaps[f"w1{l}"] = wpack(A(lp["w1"]).astype(np.float32), NKE)
        wmaps[f"w2{l}"] = wpack(A(lp["w2"]).astype(np.float32), NFF)
    whe = np.concatenate([A(p["gen_aob_w"]), A(p["gen_atom_w"]),
                          A(p["edge_w0"]), A(p["edge_w1"])], axis=0).astype(np.float32)
    wmaps["whe"] = wpack(whe, NKE)

    in_maps = []
    for c in range(NCORES):
        m = dict(wmaps)
        m["x0"] = x0_tok[c]
        m["x0T"] = x0T[c]
        m["t0"] = t0_sh[c]
        m["bias"] = bias_sh[c]
        in_maps.append(m)
    return in_maps, aqs


def _host_post(results, aqs):
    heads = []
    edges = []
    for c in range(NCORES):
        hd = results[c]["head_out"]                      # [128, NTOK, 45]
        hd = hd.transpose(1, 0, 2).reshape(TP, 45)[:T]   # (1600,45)
        heads.append(hd.reshape(NB, S, 45))
        ed = results[c]["edge_out"]                      # [NB,6,100,2,S]
        # logits_[b, s=st*100+p, t, v] = ed[b, v, p, st, t]
        edges.append(ed.transpose(0, 3, 2, 4, 1).reshape(NB, S, S, 6))
    head = np.concatenate(heads, 0)                      # (B,S,45)
    logits_ = np.concatenate(edges, 0).astype(np.float32)  # (B,S,S,6)

    aob_logits = np.ascontiguousarray(head[:, :, :5]).astype(np.float32)
    atom_logits = np.ascontiguousarray(head[:, :, 5:]).astype(np.float32)
    aob_logits[:, :, 0] = -np.inf
    aob_logits[:, :, 1] = -np.inf
    aob_logits[:, 0, 4] = -np.inf

    edge = np.full((B, S, 50, 6), -np.inf, np.float32)
    bi = np.arange(B)[:, None, None]
    si = np.arange(S)[None, :, None]
    edge[bi, si, aqs] = logits_
    edge[:, :, 0, :] = -np.inf
    edge = edge.reshape(B, S, 300)
    return aob_logits, atom_logits, edge


TRACE = False       # set True (e.g. from test.py) to capture a neuron profile


def kernel(**inputs):
    from concourse.bass_utils import run_bass_kernel_spmd
    if "nc" not in _CACHE:
        _CACHE["nc"] = _build_nc()
    nc = _CACHE["nc"]
    in_maps, aqs = _host_pre(inputs)
    res = run_bass_kernel_spmd(nc, in_maps, list(range(NCORES)), trace=TRACE)
    _CACHE["last"] = res
    return _host_post(res.results, aqs)
